# revision 13
# baseline (speedup 1.0000x reference)
"""Trainium2 Bass kernel for nn_Attention_45183055954094.

Cosine-similarity attention (temp=30) over 64 independent instances of
1024 tokens x 128 channels, with shared QK projection to head dim 32,
residual, and InstanceL2Norm. Data-parallel over the 64 instances across
8 NeuronCores (8 instances per core).

Per-instance algorithm (all on-chip, c-major layouts, fp32r matmuls):
  QT_raw (32,1024) = Wt^T @ f1r               (bias b1 = W@t_pos1 + b folded
  KT_raw (32,1024) = Wt^T @ f2r                into Square / affine_mul ops)
  B_q (32,1024) = 1/sqrt(bcast(colsum((QT_raw+b1)^2)))   via ones-matmul
  QT_n = (QT_raw + b1) * B_q ; KT_n likewise  (unit q/k vectors)
  per k-tile j: S_T = KT_n[:,j]^T @ QT_n ; E_j = exp(30 * S_T)   [f32r]
  Z (1,1024)   += ones^T @ E_j ;  AV (128,1024) += X2T_j^T @ E_j
  R = f2 + 2*t_pos2 + AV * bcast(1/Z)
  out = R * 8/sqrt(sum(R^2) + 1e-5)
"""

import sys

for _p in ("/opt/trn_rl_repo", "/root/.axon_site/_ro/trn_rl_repo"):
    if _p not in sys.path:
        sys.path.insert(0, _p)

import numpy as np

B, N, C, H, W = 16, 4, 128, 32, 32
HW = H * W           # 1024 tokens
NI = B * N           # 64 instances
NCORES = 8
IPC = NI // NCORES   # 8 instances per core
GRP = 4              # instances per ACT-table phase group

_CACHE = {}


def _build(ipc=IPC, grp=GRP):
    import concourse.bass as bass
    import concourse.tile as tile
    from concourse import bacc, mybir
    from concourse.bass import ts

    f32 = mybir.dt.float32
    f32r = mybir.dt.float32r
    AF = mybir.ActivationFunctionType

    nc = bacc.Bacc("TRN2", target_bir_lowering=False, debug=False)

    f1_d = nc.dram_tensor("f1", [ipc, C, HW], f32, kind="ExternalInput").ap()
    f2_d = nc.dram_tensor("f2", [ipc, C, HW], f32, kind="ExternalInput").ap()
    wt_d = nc.dram_tensor("wt", [C, C], f32, kind="ExternalInput").ap()
    b1_d = nc.dram_tensor("b1", [C, 1], f32, kind="ExternalInput").ap()
    b2_d = nc.dram_tensor("b2", [C, 1], f32, kind="ExternalInput").ap()
    blk_d = nc.dram_tensor("blk", [C, C], f32, kind="ExternalInput").ap()
    id_d = nc.dram_tensor("ident", [C, C], f32, kind="ExternalInput").ap()
    t2_d = nc.dram_tensor("t2x2", [C, 1], f32, kind="ExternalInput").ap()
    out_d = nc.dram_tensor("out", [ipc, C, HW], f32, kind="ExternalOutput").ap()

    with tile.TileContext(nc) as tc:
        import contextlib

        with contextlib.ExitStack() as ctx:
            consts = ctx.enter_context(tc.tile_pool(name="consts", bufs=1))
            fin = ctx.enter_context(tc.tile_pool(name="fin", bufs=2))
            frp = ctx.enter_context(tc.tile_pool(name="frp", bufs=3))
            f2p = ctx.enter_context(tc.tile_pool(name="f2p", bufs=grp))
            qtnp = ctx.enter_context(tc.tile_pool(name="qtnp", bufs=grp))
            ktp = ctx.enter_context(tc.tile_pool(name="ktp", bufs=grp))
            x2tp = ctx.enter_context(tc.tile_pool(name="x2tp", bufs=grp))
            sqp = ctx.enter_context(tc.tile_pool(name="sqp", bufs=2))
            binvp = ctx.enter_context(tc.tile_pool(name="binvp", bufs=2))
            ep = ctx.enter_context(tc.tile_pool(name="ep", bufs=10))
            zp = ctx.enter_context(tc.tile_pool(name="zp", bufs=2))
            bzp = ctx.enter_context(tc.tile_pool(name="bzp", bufs=2))
            t1p = ctx.enter_context(tc.tile_pool(name="t1p", bufs=2))
            rp = ctx.enter_context(tc.tile_pool(name="rp", bufs=grp))
            scrp = ctx.enter_context(tc.tile_pool(name="scrp", bufs=2))
            colp = ctx.enter_context(tc.tile_pool(name="colp", bufs=2 * grp))
            op = ctx.enter_context(tc.tile_pool(name="op", bufs=2))
            ps = ctx.enter_context(tc.tile_pool(name="ps", bufs=4, space="PSUM"))

            # ---- constants ----
            wt_sb = consts.tile([C, C], f32, tag="wt")
            nc.sync.dma_start(wt_sb[:], wt_d[:])
            wt_r = consts.tile([C, C], f32r, tag="wtr")
            nc.vector.tensor_copy(wt_r[:], wt_sb[:])
            blk_sb = consts.tile([C, C], f32, tag="blk")
            nc.sync.dma_start(blk_sb[:], blk_d[:])
            blk_r = consts.tile([C, C], f32r, tag="blkr")
            nc.vector.tensor_copy(blk_r[:], blk_sb[:])
            t2_sb = consts.tile([C, 1], f32, tag="t2")
            nc.sync.dma_start(t2_sb[:], t2_d[:])
            b1_sb = consts.tile([C, 1], f32, tag="b1")
            nc.sync.dma_start(b1_sb[:], b1_d[:])
            b2_sb = consts.tile([C, 1], f32, tag="b2")
            nc.sync.dma_start(b2_sb[:], b2_d[:])
            id_sb = consts.tile([C, C], f32, tag="id")
            nc.sync.dma_start(id_sb[:], id_d[:])
            id_r = consts.tile([C, C], f32r, tag="idr")
            nc.vector.tensor_copy(id_r[:], id_sb[:])

            ones32_f = consts.tile([32, 32], f32, tag="ones32f")
            nc.vector.memset(ones32_f[:], 1.0)
            ones32_r = consts.tile([32, 32], f32r, tag="ones32r")
            nc.vector.tensor_copy(ones32_r[:], ones32_f[:])

            ones128_f = consts.tile([C, 1], f32, tag="ones128f")
            nc.vector.memset(ones128_f[:], 1.0)
            ones128_r = consts.tile([C, 1], f32r, tag="ones128r")
            nc.vector.tensor_copy(ones128_r[:], ones128_f[:])

            onesrow_f = consts.tile([1, C], f32, tag="onesrowf")
            nc.vector.memset(onesrow_f[:], 1.0)
            onesrow_r = consts.tile([1, C], f32r, tag="onesrowr")
            nc.vector.tensor_copy(onesrow_r[:], onesrow_f[:])

            gbias_sb = consts.tile([1, 1], f32, tag="gbias")
            nc.vector.memset(gbias_sb[:], 1e-5 / 64.0)

            kt_sbs = {}
            qtn_sbs = {}
            x2t_sbs = {}
            f2_sbs = {}
            r_sbs = {}
            ssq_cols = {}

            def qk_proj(f_r):
                # 4x-replicated projection: wt_r holds [wt|wt|wt|wt]
                psum_p = ps.tile([C, HW], f32, tag="ps")
                for h in range(2):
                    sl = ts(h, 512)
                    nc.tensor.matmul(psum_p[:, sl], wt_r[:], f_r[:, sl],
                                     start=True, stop=True)
                return psum_p

            def qk_norm(psum_p, bias_sb, out_pool, tag):
                sq = sqp.tile([C, HW], f32r, tag="sq")
                nc.scalar.activation(sq[:], psum_p[:], AF.Square,
                                     bias=bias_sb[:])
                psum_b = ps.tile([C, HW], f32, tag="ps")
                for h in range(2):
                    sl = ts(h, 512)
                    nc.tensor.matmul(psum_b[:, sl], blk_r[:], sq[:, sl],
                                     start=True, stop=True)
                b_inv = binvp.tile([C, HW], f32r, tag="binv")
                nc.scalar.activation(b_inv[:], psum_b[:],
                                     AF.Abs_reciprocal_sqrt)
                outn = out_pool.tile([C, HW], f32r, tag=tag)
                junk_col = colp.tile([C, 1], f32, tag="junk")
                nc.vector.affine_mul_reduce(out=outn[:], accum_out=junk_col[:],
                                            in0=psum_p[:], in1=b_inv[:],
                                            scale=1.0, bias=bias_sb[:])
                return outn

            def phase_a(i):
                """DMA + f32r conversion + projections + transposes + norms.
                Transposes sit between projections and norm-matmuls as PE
                filler while ACT computes the squares."""
                f1_sb = fin.tile([C, HW], f32, tag="fin")
                nc.sync.dma_start(f1_sb[:], f1_d[i])
                f2_sb = f2p.tile([C, HW], f32, tag="f2")
                nc.sync.dma_start(f2_sb[:], f2_d[i])
                f2_sbs[i] = f2_sb
                f1_r = frp.tile([C, HW], f32r, tag="fr")
                nc.vector.tensor_copy(f1_r[:], f1_sb[:])
                f2_r = frp.tile([C, HW], f32r, tag="fr")
                nc.vector.tensor_copy(f2_r[:], f2_sb[:])

                pq = qk_proj(f1_r)
                pk = qk_proj(f2_r)
                psum_t = ps.tile([C, HW], f32, tag="ps")
                for j in range(8):
                    nc.tensor.transpose(psum_t[:, ts(j, C)].bitcast(f32r),
                                        f2_r[:, ts(j, C)], id_r[:])
                x2t = x2tp.tile([C, HW], f32r, tag="x2t")
                nc.vector.tensor_copy(x2t[:], psum_t[:])
                x2t_sbs[i] = x2t
                qtn_sbs[i] = qk_norm(pq, b1_sb, qtnp, "qtn")
                kt_sbs[i] = qk_norm(pk, b2_sb, ktp, "kt")

            state = {}

            def phase_b_head(i):
                """S_T + exp pipeline, then dense AV and Z blocks."""
                qtn, kt, x2t = qtn_sbs[i], kt_sbs[i], x2t_sbs[i]

                def st_mms(j):
                    rg = 32 * (j % 2)
                    psum_s = ps.tile([C, HW], f32, tag="ps")
                    for h in range(2):
                        sl = ts(h, 512)
                        nc.tensor.matmul(psum_s[:, sl],
                                         kt[rg:rg + 32, ts(j, C)],
                                         qtn[rg:rg + 32, sl],
                                         start=True, stop=True)
                    return psum_s

                s_tiles = {0: st_mms(0), 1: st_mms(1), 2: st_mms(2)}
                e_sbs = []
                for j in range(8):
                    e_sb = ep.tile([C, HW], f32r, tag="e")
                    nc.scalar.activation(e_sb[:], s_tiles.pop(j)[:], AF.Exp,
                                         scale=30.0)
                    e_sbs.append(e_sb)
                    if j + 3 < 8:
                        s_tiles[j + 3] = st_mms(j + 3)
                # dense AV block
                psum_av = ps.tile([C, HW], f32, tag="ps")
                for j in range(8):
                    for h in range(2):
                        sl = ts(h, 512)
                        nc.tensor.matmul(psum_av[:, sl], x2t[:, ts(j, C)],
                                         e_sbs[j][:, sl],
                                         start=(j == 0), stop=(j == 7))
                # dense Z block
                psum_z = ps.tile([1, HW], f32, tag="ps")
                for j in range(8):
                    for h in range(2):
                        sl = ts(h, 512)
                        nc.tensor.matmul(psum_z[0:1, sl], ones128_r[:],
                                         e_sbs[j][:, sl],
                                         start=(j == 0), stop=(j == 7))
                z_sb = zp.tile([1, HW], f32r, tag="z")
                nc.vector.tensor_copy(z_sb[:], psum_z[:])
                state[i] = (psum_av, z_sb)

            def phase_b_tail(i):
                psum_av, z_sb = state.pop(i)
                psum_bz = ps.tile([C, HW], f32, tag="ps")
                for h in range(2):
                    sl = ts(h, 512)
                    nc.tensor.matmul(psum_bz[:, sl], onesrow_r[:],
                                     z_sb[:, sl], start=True, stop=True)
                bz_sb = bzp.tile([C, HW], f32, tag="bz")
                nc.vector.reciprocal_approx_fast(bz_sb[:], psum_bz[:])
                t1 = t1p.tile([C, HW], f32, tag="t1")
                nc.vector.tensor_mul(t1[:], psum_av[:], bz_sb[:])
                r_sb = rp.tile([C, HW], f32, tag="r")
                nc.vector.affine_then_add(r_sb[:], f2_sbs[i], t1[:],
                                          scale=1.0, bias=t2_sb[:])
                r_sbs[i] = r_sb
                scr = scrp.tile([C, HW], f32, tag="scr")
                ssq_col = colp.tile([C, 1], f32, tag="ssqc")
                nc.vector.tensor_mul(scr[:], r_sb[:], r_sb[:])
                nc.vector.reduce_sum(ssq_col[:], scr[:],
                                     axis=mybir.AxisListType.X)
                ssq_cols[i] = ssq_col

            def phase_c(i):
                psum_g = ps.tile([1, 1], f32, tag="ps")
                nc.tensor.matmul(psum_g[:], ones128_f[:], ssq_cols[i][:],
                                 start=True, stop=True)
                g_sb = colp.tile([1, 1], f32, tag="g")
                # 8/sqrt(ssq + 1e-5) = 1/sqrt(ssq/64 + 1e-5/64)
                nc.scalar.activation(g_sb[:], psum_g[:], AF.Abs_reciprocal_sqrt,
                                     scale=1.0 / 64.0, bias=gbias_sb[:])
                psum_gc = ps.tile([C, 1], f32, tag="ps")
                nc.tensor.matmul(psum_gc[:], onesrow_f[:], g_sb[:],
                                 start=True, stop=True)
                g_col = colp.tile([C, 1], f32, tag="gc")
                nc.vector.tensor_copy(g_col[:], psum_gc[:])
                o_sb = op.tile([C, HW], f32, tag="o")
                nc.vector.tensor_scalar_mul(o_sb[:], r_sbs[i][:], g_col[:])
                nc.sync.dma_start(out_d[i], o_sb[:])

            ngroups = (ipc + grp - 1) // grp
            groups = [range(g * grp, min((g + 1) * grp, ipc))
                      for g in range(ngroups)]
            for g in range(ngroups):
                for n, i in enumerate(groups[g]):
                    phase_a(i)
                    if n == 0 and g > 0:
                        for ii in groups[g - 1]:
                            phase_c(ii)
                prev = None
                for i in groups[g]:
                    phase_b_head(i)
                    if prev is not None:
                        phase_b_tail(prev)
                    prev = i
                phase_b_tail(prev)
            for i in groups[-1]:
                phase_c(i)

    nc.compile()
    return nc


def kernel(**inputs) -> np.ndarray:
    from concourse import bass_utils

    f_list1 = np.asarray(inputs["f_list1"], dtype=np.float32)
    f_list2 = np.asarray(inputs["f_list2"], dtype=np.float32)
    t_pos1 = np.asarray(inputs["t_pos1"], dtype=np.float32).reshape(C)
    t_pos2 = np.asarray(inputs["t_pos2"], dtype=np.float32).reshape(C)
    W_qk_w = np.asarray(inputs["W_qk_w"], dtype=np.float32)
    W_qk_b = np.asarray(inputs["W_qk_b"], dtype=np.float32)

    # fold t_pos into the projection biases: q = W @ (x + t1) + b
    b1v = (W_qk_w @ t_pos1 + W_qk_b).astype(np.float32).reshape(32, 1)
    b2v = (W_qk_w @ t_pos2 + W_qk_b).astype(np.float32).reshape(32, 1)
    b1 = np.tile(b1v, (4, 1))                           # (128, 1)
    b2 = np.tile(b2v, (4, 1))
    ident = np.eye(C, dtype=np.float32)
    wt = np.ascontiguousarray(np.tile(W_qk_w.T, (1, 4)))  # (128, 128)
    blk = np.kron(np.eye(4, dtype=np.float32),
                  np.ones((32, 32), dtype=np.float32))    # (128, 128)
    t2x2 = (2.0 * t_pos2).astype(np.float32).reshape(C, 1)

    f1 = np.ascontiguousarray(f_list1.reshape(NI, C, HW))
    f2 = np.ascontiguousarray(f_list2.reshape(NI, C, HW))

    if "nc" not in _CACHE:
        _CACHE["nc"] = _build()
    nc = _CACHE["nc"]

    in_maps = []
    for c in range(NCORES):
        sl = slice(c * IPC, (c + 1) * IPC)
        in_maps.append({
            "f1": np.ascontiguousarray(f1[sl]),
            "f2": np.ascontiguousarray(f2[sl]),
            "wt": wt, "b1": b1, "b2": b2, "t2x2": t2x2, "ident": ident,
            "blk": blk,
        })

    res = bass_utils.run_bass_kernel_spmd(nc, in_maps,
                                          core_ids=list(range(NCORES)))
    out = np.empty((NI, C, HW), dtype=np.float32)
    for c in range(NCORES):
        out[c * IPC:(c + 1) * IPC] = res.results[c]["out"]
    return out.reshape(NI, C, H, W)
